# revision 12
# baseline (speedup 1.0000x reference)
"""AttentiveReduce Trainium2 kernel (v2: bf16 + FWL-stationary matmuls).

Reference computation (B=32, L=4096, D=768, H=8, Dh=96):
    xn   = LayerNorm(x; gamma1, beta1)            [B,L,D]
    kv   = xn @ w_kv.T ; k, v = split(kv)         [B,L,D] each
    dots = einsum('hd,blhd->bhl', q, k) * Dh^-0.5
    attn = softmax(dots, axis=-1)
    out  = einsum('bhl,blhd->bhd', attn, v) -> [B,D]
    out  = LayerNorm(out; gamma2, beta2)

Same algebraic restructuring as v1 (fold q into Wk on host, pool x then
project with Wv on host), but the device pipeline is rebuilt around two
facts measured on this part:
  - Fast Weight Load: a 128-column bf16 stationary loads in ~32 PE
    cycles, so matmuls with x as the *stationary* and a tiny moving
    operand (g: 9 cols, u: 8 cols, ones: 1 col) cost ~15-27 ns each.
  - bf16 inputs halve HBM traffic (24 MiB/core), the memory roofline.

Per 128-token tile (bf16 everywhere, fp32 PSUM):
  - 6 PE transposes x-chunk -> x^T in PSUM (bf16 out)
  - DVE/ACT copy x^T PSUM->SBUF (xt)
  - dots:  lhsT=xt chunk (FWL), rhs=g[128d,9]  -> y[128t,9] (token-major!)
  - ssq:   either ACT Square+accum on xe, or square xt on Pool/DVE ->
           sqT, then PE-reduce: lhsT=sqT chunk (FWL), rhs=ones[128,1]
  - P1:    lhsT=xe chunk (FWL), rhs=u[128t,8] -> P1^T[128d,8] accum/batch
  - UZ:    lhsT=[mu,sigma] (2 cols), rhs=u -> [2,8] accum/batch
Stats (ln/exp on one ACT table set), softmax weights u = exp(r*(y-mu*s)
+ c)*r as in v1.  Host epilogue unchanged (tiny [32,8,768] math).

Sharding: data-parallel over batch: 8 cores x 4 batches.
"""

import sys

if "/opt/trn_rl_repo" not in sys.path:
    sys.path.insert(0, "/opt/trn_rl_repo")

import numpy as np
import ml_dtypes

import concourse.bacc as bacc
import concourse.tile as tile
from concourse import bass_utils, mybir


f32 = mybir.dt.float32
bf16 = mybir.dt.bfloat16
AF = mybir.ActivationFunctionType
ALU = mybir.AluOpType

B, L, D, H, Dh = 32, 4096, 768, 8, 96
EPS = 1e-5
NCORES = 8
BPC = B // NCORES  # batches per core
PT = 128           # tokens per partition tile
MACRO = 512        # tokens per macro tile (4 p-tiles)
NPT = MACRO // PT  # 4
NC6 = D // 128     # 6 d-chunks of 128
YW = 12            # y psum row: 9 (dots+mu) + 1 (ssq) + pad
P1W = 56           # p1 psum: 48 (6 chunks x 8 heads) + 8 (uz cols)

# ssq engine per global p-tile index (cycle): "pe_pool" = square on Pool +
# PE ones-reduce; "pe_dve" = square on DVE + PE reduce; "act" = fused
# Square+accum on ACT directly from xe.
SSQ_CYCLE = (
    "pe_dve", "pe_pool", "act", "pe_dve", "pe_pool", "pe_dve", "act",
    "pe_pool", "pe_dve", "pe_pool", "pe_dve", "pe_pool", "act", "pe_dve",
    "pe_pool", "pe_dve",
)
# x^T PSUM->SBUF copy engine per global p-tile (cycle)
CP_CYCLE = (
    "dve", "act", "dve", "act", "dve", "act", "dve", "dve", "act", "dve",
    "act", "dve", "act", "dve", "act", "dve",
)


def _build(bpc, nmac, use_c):
    nc = bacc.Bacc("TRN2", target_bir_lowering=False, debug=False)

    x_in = nc.dram_tensor("x", [bpc, nmac, 128, NPT * D], bf16, kind="ExternalInput")
    g_in = nc.dram_tensor("gmat", [128, NC6, 16], bf16, kind="ExternalInput")
    sc_in = nc.dram_tensor("scvec", [128, 16], f32, kind="ExternalInput")
    id_in = nc.dram_tensor("ident", [128, 128], bf16, kind="ExternalInput")
    p1_out = nc.dram_tensor("p1out", [bpc, 128, P1W], f32, kind="ExternalOutput")

    with tile.TileContext(nc) as tc:
        with (
            tc.tile_pool(name="singles", bufs=1) as singles,
            tc.tile_pool(name="xe", bufs=nmac + 5) as xe_pool,
            tc.tile_pool(name="xt", bufs=10) as xt_pool,
            tc.tile_pool(name="sq", bufs=10) as sq_pool,
            tc.tile_pool(name="yb", bufs=2) as yb_pool,
            tc.tile_pool(name="uw", bufs=2) as uw_pool,
            tc.tile_pool(name="st", bufs=2) as st_pool,
            tc.tile_pool(name="junk", bufs=2) as junk_pool,
            tc.tile_pool(name="osb", bufs=2) as osb_pool,
            tc.tile_pool(name="xtp", bufs=5, space="PSUM") as xtp_pool,
            tc.tile_pool(name="yp", bufs=2, space="PSUM") as yp_pool,
            tc.tile_pool(name="p1p", bufs=1, space="PSUM") as p1p_pool,
        ):
            id_sb = singles.tile([128, 128], bf16)
            nc.sync.dma_start(out=id_sb, in_=id_in[:, :])
            g_sb = singles.tile([128, NC6, 16], bf16)
            nc.sync.dma_start(out=g_sb, in_=g_in[:, :, :])
            sc_sb = singles.tile([128, 16], f32)
            nc.sync.dma_start(out=sc_sb, in_=sc_in[:, :])
            eps_t = singles.tile([128, 1], f32)
            nc.vector.memset(eps_t, EPS)
            ones_t = singles.tile([128, 1], bf16)
            nc.vector.memset(ones_t, 1.0)

            s_bc = (
                sc_sb[:, 0:8].unsqueeze(1).unsqueeze(1).to_broadcast([128, nmac, NPT, 8])
            )
            c_bc = (
                sc_sb[:, 8:16].unsqueeze(1).unsqueeze(1).to_broadcast([128, nmac, NPT, 8])
            )

            def phase_a_produce(b, m, yb):
                """DMA + transposes + x^T copies + squares for macro m."""
                xe = xe_pool.tile([128, NPT, D], bf16, tag="xe")
                nc.sync.dma_start(
                    out=xe.rearrange("p pt d -> p (pt d)"), in_=x_in[b, m, :, :]
                )
                xts, sqs = [], []
                for pt in range(NPT):
                    ptg = m * NPT + pt
                    xtp = xtp_pool.tile([128, NC6, 128], bf16, tag="xtp")
                    for c in range(NC6):
                        nc.tensor.transpose(
                            xtp[:, c, :], xe[:, pt, c * 128 : (c + 1) * 128], id_sb
                        )
                    xt = xt_pool.tile([128, NC6, 128], bf16, tag="xt")
                    cp = CP_CYCLE[ptg % len(CP_CYCLE)]
                    if cp == "dve":
                        nc.vector.tensor_copy(xt, xtp)
                    elif cp == "dma":
                        nc.scalar.dma_start(out=xt, in_=xtp)
                    else:
                        nc.scalar.copy(xt, xtp)
                    xts.append(xt)

                    mode = SSQ_CYCLE[ptg % len(SSQ_CYCLE)]
                    sqT = None
                    if mode == "pe_pool":
                        sqT = sq_pool.tile([128, NC6, 128], bf16, tag="sq")
                        nc.gpsimd.tensor_mul(sqT, xt, xt)
                    elif mode == "pe_dve":
                        sqT = sq_pool.tile([128, NC6, 128], bf16, tag="sq")
                        nc.vector.tensor_mul(sqT, xt, xt)
                    else:
                        junk = junk_pool.tile([128, D], bf16, tag="junk")
                        nc.scalar.activation(
                            junk, xe[:, pt, :], AF.Square,
                            accum_out=yb[:, m, pt, 9:10],
                        )
                    sqs.append(sqT)
                return xe, xts, sqs

            def phase_a_consume(m, yb, xts, sqs):
                """dots + ssq matmuls for macro m (one macro behind produce)."""
                yp = yp_pool.tile([128, NPT, YW], f32, tag="yp")
                for pt in range(NPT):
                    first = pt == 0
                    last = pt == NPT - 1
                    for c in range(NC6):
                        nc.tensor.matmul(
                            yp[:, pt, 0:9],
                            xts[pt][:, c, :],
                            g_sb[:, c, 0:9],
                            start=(first and c == 0),
                            stop=False,
                        )
                    if sqs[pt] is not None:
                        for c in range(NC6):
                            nc.tensor.matmul(
                                yp[:, pt, 9:10],
                                sqs[pt][:, c, :],
                                ones_t,
                                start=False,
                                stop=(last and c == NC6 - 1),
                            )
                    elif last:
                        nc.tensor.matmul(
                            yp[:, pt, 10:11], xts[0][:, 0, :], ones_t,
                            start=False, stop=True,
                        )
                nc.vector.tensor_copy(yb[:, m, :, 0:9], yp[:, :, 0:9])
                for pt in range(NPT):
                    ptg = m * NPT + pt
                    if SSQ_CYCLE[ptg % len(SSQ_CYCLE)] != "act":
                        nc.vector.tensor_copy(
                            yb[:, m, pt, 9:10], yp[:, pt, 9:10]
                        )

            def phase_b(yb, musig):
                """Per-batch stats: var -> r, sigma; build musig (bf16)."""
                mu_ap = yb[:, :, :, 8:9]
                ssq_ap = yb[:, :, :, 9:10]
                m2 = st_pool.tile([128, nmac, NPT, 1], f32, tag="m2")
                nc.vector.tensor_mul(m2, mu_ap, mu_ap)
                var = st_pool.tile([128, nmac, NPT, 1], f32, tag="var")
                nc.vector.scalar_tensor_tensor(
                    var, ssq_ap, 1.0 / D, m2, op0=ALU.mult, op1=ALU.subtract
                )
                lnv = st_pool.tile([128, nmac * NPT], f32, tag="lnv")
                nc.scalar.activation(
                    lnv, var.rearrange("p m q o -> p (m q o)"), AF.Ln,
                    bias=eps_t[:, :],
                )
                r_all = st_pool.tile([128, nmac * NPT], f32, tag="r")
                nc.scalar.activation(r_all, lnv, AF.Exp, scale=-0.5)
                sg_all = st_pool.tile([128, nmac * NPT], f32, tag="sg")
                nc.scalar.activation(sg_all, lnv, AF.Exp, scale=0.5)
                # musig[p, m, pt, 0:2] = (mu, sigma) in bf16
                nc.vector.tensor_copy(
                    musig[:, :, :, 0:1], mu_ap
                )
                nc.vector.tensor_copy(
                    musig[:, :, :, 1:2],
                    sg_all.rearrange("p (m q) -> p m q", q=NPT).unsqueeze(3),
                )
                return r_all

            def phase_c_weights(yb, musig, r_all):
                """u = exp(r*(y - mu*s) + c) * r for the whole batch."""
                r_bc = (
                    r_all[:]
                    .rearrange("p (m q) -> p m q", q=NPT)
                    .unsqueeze(3)
                    .to_broadcast([128, nmac, NPT, 8])
                )
                mu_bc = yb[:, :, :, 8:9].to_broadcast([128, nmac, NPT, 8])
                prod = uw_pool.tile([128, nmac, NPT, 8], f32, tag="prod")
                nc.vector.tensor_mul(prod, mu_bc, s_bc)
                diff = uw_pool.tile([128, nmac, NPT, 8], f32, tag="diff")
                nc.vector.tensor_sub(diff, yb[:, :, :, 0:8], prod)
                arg = uw_pool.tile([128, nmac, NPT, 8], f32, tag="arg")
                nc.vector.tensor_mul(arg, diff, r_bc)
                if use_c:
                    arg2 = uw_pool.tile([128, nmac, NPT, 8], f32, tag="arg2")
                    nc.vector.tensor_add(arg2, arg, c_bc)
                    arg = arg2
                w_t = uw_pool.tile([128, nmac, NPT, 8], f32, tag="w")
                nc.scalar.activation(w_t, arg, AF.Exp)
                u_t = uw_pool.tile([128, nmac, NPT, 8], bf16, tag="u")
                nc.vector.tensor_mul(u_t, w_t, r_bc)
                return u_t

            def phase_c(b, m, musig, u_t, xe, p1z, first_m, last_m):
                """P1/UZ matmuls for macro m."""
                for pt in range(NPT):
                    first = first_m and pt == 0
                    last = last_m and pt == NPT - 1
                    for c in range(NC6):
                        nc.tensor.matmul(
                            p1z[:, c * 8 : (c + 1) * 8],
                            xe[:, pt, c * 128 : (c + 1) * 128],
                            u_t[:, m, pt, :],
                            start=(first and c == 0),
                            stop=False,
                        )
                    nc.tensor.matmul(
                        p1z[0:2, 48:56],
                        musig[:, m, pt, 0:2],
                        u_t[:, m, pt, :],
                        start=False,
                        stop=last,
                    )

            for b in range(bpc):
                yb = yb_pool.tile([128, nmac, NPT, YW], f32, tag="yb")
                musig = st_pool.tile([128, nmac, NPT, 4], bf16, tag="musig")
                xes = []
                for m in range(nmac):
                    produced = phase_a_produce(b, m, yb)
                    xes.append(produced[0])
                    phase_a_consume(m, yb, produced[1], produced[2])
                r_all = phase_b(yb, musig)
                u_t = phase_c_weights(yb, musig, r_all)
                p1z = p1p_pool.tile([128, P1W], f32, tag="p1z")
                for m in range(nmac):
                    phase_c(
                        b, m, musig, u_t, xes[m], p1z,
                        first_m=(m == 0), last_m=(m == nmac - 1),
                    )
                p1s = osb_pool.tile([128, P1W], f32, tag="p1s")
                nc.vector.tensor_copy(p1s, p1z)
                nc.sync.dma_start(out=p1_out[b], in_=p1s)

    return nc


_CACHE = {}


def _get_compiled(bpc, nmac, use_c):
    key = (bpc, nmac, use_c)
    if key not in _CACHE:
        nc = _build(bpc, nmac, use_c)
        nc.compile()
        _CACHE[key] = nc
    return _CACHE[key]


def _host_params(w_kv, query, gamma1, beta1):
    scale = Dh**-0.5
    wk = w_kv[:D]
    qw = (query.reshape(H, Dh)[:, :, None] * wk.reshape(H, Dh, D)).sum(1) * scale
    a = gamma1[None, :] * qw                    # [H, D]
    s = a.sum(-1).astype(np.float32)            # [H]
    c = (beta1[None, :] * qw).sum(-1).astype(np.float32)

    g = np.zeros((D, 16), np.float32)
    g[:, :8] = a.T
    g[:, 8] = 1.0 / D
    gb = g.astype(ml_dtypes.bfloat16).reshape(NC6, 128, 16).transpose(1, 0, 2)
    gb = np.ascontiguousarray(gb)
    scv = np.zeros((128, 16), np.float32)
    scv[:, 0:8] = s[None, :]
    scv[:, 8:16] = c[None, :]
    ident = np.eye(128, dtype=ml_dtypes.bfloat16)
    return gb, scv, ident, c


def kernel(x, w_kv, query, gamma1, beta1, gamma2, beta2, _run_opts=None):
    x = np.asarray(x, np.float32)
    w_kv = np.asarray(w_kv, np.float32)
    query = np.asarray(query, np.float32)
    gamma1 = np.asarray(gamma1, np.float32)
    beta1 = np.asarray(beta1, np.float32)
    gamma2 = np.asarray(gamma2, np.float32)
    beta2 = np.asarray(beta2, np.float32)

    gb, scv, ident, c = _host_params(w_kv, query, gamma1, beta1)
    use_c = not np.allclose(c, 0.0)
    nc = _get_compiled(BPC, L // MACRO, use_c)
    # reorder tokens so each SBUF partition line is one contiguous 6KB run:
    # x[b, m*512 + pt*128 + p, d] -> xs[b, m, p, pt*768 + d]
    xb = x.astype(ml_dtypes.bfloat16)
    xs = np.ascontiguousarray(
        xb.reshape(B, L // MACRO, NPT, 128, D).transpose(0, 1, 3, 2, 4)
    ).reshape(B, L // MACRO, 128, NPT * D)
    in_maps = [
        {"x": xs[i * BPC : (i + 1) * BPC], "gmat": gb, "scvec": scv, "ident": ident}
        for i in range(NCORES)
    ]
    res = bass_utils.run_bass_kernel_spmd(
        nc, in_maps, core_ids=list(range(NCORES)), **(_run_opts or {})
    )
    p1 = np.concatenate([res.results[i]["p1out"] for i in range(NCORES)], axis=0)

    out = _epilogue(p1, w_kv, gamma1, beta1, gamma2, beta2)
    if _run_opts:
        return out, res
    return out


def _epilogue(p1, w_kv, gamma1, beta1, gamma2, beta2):
    """pooled -> v-projection -> final LayerNorm, on [32,8,768]-sized data.

    p1 layout: [B, 128, 56]: cols c*8+h = P1^T[d=c*128+dd, h];
    col 48+h rows 0/1 = U[h] / Z[h].
    """
    P1 = p1[:, :, 0:48].reshape(B, 128, NC6, H).transpose(0, 3, 2, 1).reshape(B, H, D)
    U = p1[:, 0, 48:56]                        # [B, H]
    Z = p1[:, 1, 48:56]                        # [B, H]
    pooled = gamma1[None, None, :] * (P1 - U[:, :, None]) / Z[:, :, None]
    pooled += beta1[None, None, :]
    wv = w_kv[D:].reshape(H, Dh, D)
    out0 = np.einsum("bhd,hjd->bhj", pooled, wv, optimize=True).reshape(B, D)
    mu = out0.mean(-1, keepdims=True)
    var = out0.var(-1, keepdims=True)
    out = (out0 - mu) / np.sqrt(var + EPS) * gamma2[None, :] + beta2[None, :]
    return out.astype(np.float32)
